# revision 10
# baseline (speedup 1.0000x reference)
"""Trainium2 Bass kernel for nn_AttentionBlock (B=4, C=256, L=4096, GROUPS=8).

y = x + out_w @ attn(group_norm(x)) + out_b  with single-head attention over
the full length L, channels-as-features (c=256), softmax over key positions.

Sharding: 8 cores = (batch, L/2-half) pairs. Each core receives the full
x[b] (needed for GroupNorm stats and for K/V over the whole sequence), with
columns rotated so its own half sits in [:, :2048]; attention is permutation
invariant over key positions, so the rotation only needs to be consistent
between K/V and the GN stats (it is: GN stats are order invariant).

Per-core pipeline:
  1. GroupNorm stats: per-channel sum/sumsq (free-dim reduce), group-reduce
     over channels via a tiny selector matmul, broadcast back via a second
     tiny matmul; h = s*x + t applied per 512-col block.
  2. QKV: q = wq @ h (own half only), k = wk @ h (full), vT = h^T wv^T
     (computed directly in [kpos, c] layout by matmul with h as stationary).
     A ones column is appended to vT.
  3. Attention per 256-col l-block: scoresT[k, l] = k^T q via matmul (both
     operands in natural [c, *] layout), exp on ScalarE straight out of PSUM
     (no max subtraction: scores are ~N(0,1) after the 1/sqrt(c) scale, and
     exp overflow needs |s|>88), then o_u^T[l, c|sum] = expT^T @ [vT|1] —
     the ones column makes the softmax denominator fall out of the same
     accumulation. Normalize by the reciprocal of that column (per-partition
     scalar), transpose o^T -> o on the PE, project, add residual, DMA out.

Matmuls run in float32r (full-rate fp32 mode at N>=256); everything else f32.
"""

import sys
import types

sys.path.insert(0, "/opt/trn_rl_repo")
if "/root/.axon_site" not in sys.path:
    sys.path.insert(0, "/root/.axon_site")

import numpy as np

import concourse.bass as bass
import concourse.mybir as mybir
import concourse.tile as tile
from concourse.masks import make_identity

B, C, L = 4, 256, 4096
G = 8
EPS = 1e-5
HALF = L // 2  # 2048 columns per core
N_CORES = 8
CT = C // 128  # channel tiles (2)
LB = 512  # phase-2 l-block (qkv)
AB = 256  # phase-3 l-block (attention)
KT = L // 128  # key-position tiles (32)

F32 = mybir.dt.float32
F32R = mybir.dt.float32r


def _patch_tile_drain():
    """walrus in this container caps sync-waits at 2 per CTRL instruction;
    TileContext's tail drain attaches one per live semaphore. Spill the
    excess onto nops before the all-engine barrier (semantics unchanged)."""
    import bass_rust
    import concourse.tile as tile_mod

    if getattr(tile_mod.TileContext, "_drain_patch_v1", False):
        return

    def _drain_and_barrier(self, tick_clock, wait_clock):
        nc = self.nc
        drain_inst = nc.sync.drain()
        wait_clock.add_sem_waits(
            drain_inst.ins, tile_mod.ScopedClock({None: tick_clock.global_clock})
        )
        si = drain_inst.ins.sync_info
        if si is not None and si.on_wait and len(si.on_wait) > 1:
            waits = list(si.on_wait)
            si.on_wait = waits[:1]
            for i in range(1, len(waits)):
                nop = nc.sync.nop(nofuse=True, hint="drain_wait_spill")
                nsi = nop.ins.sync_info
                if nsi is None:
                    nop.ins.sync_info = bass_rust.SyncInfo(
                        on_wait=waits[i : i + 1], on_update=[]
                    )
                else:
                    nsi.on_wait = waits[i : i + 1]

        nc.all_engine_barrier()
        assert self.sems is not None
        popped = nc._tile_sem_poison_stack.pop()
        assert popped is self._sem_poison
        nc.clear_and_free_semaphores(list(self.sems.allocated().values()))
        nc.all_engine_barrier()

    tile_mod.TileContext._drain_and_barrier = _drain_and_barrier

    # This walrus also enforces the 1-wait limit on every engine
    # instruction, while Tile's add_semaphores attaches up to 4. Spill
    # extras onto same-engine nops placed immediately before (engines
    # execute their stream in order, so this is equivalent).
    orig_add = tile_mod.TileContext._add_instruction

    def _add_instruction(self, inst):
        si = inst.sync_info
        if si is not None and si.on_wait and len(si.on_wait) > 1:
            waits = list(si.on_wait)
            si.on_wait = waits[:1]
            for i in range(1, len(waits)):
                nop = mybir.InstNoOp(
                    name=f"{inst.name}-wspill{i}", text_hint="wait_spill"
                )
                nop.engine = inst.engine
                nop.sync_info = bass_rust.SyncInfo(
                    on_wait=[waits[i]], on_update=[]
                )
                orig_add(self, nop)
        orig_add(self, inst)

    tile_mod.TileContext._add_instruction = _add_instruction
    tile_mod.TileContext._drain_patch_v1 = True


def _r(ap):
    return ap.bitcast(F32R)


def build_nc():
    nc = bass.Bass("TRN2", target_bir_lowering=False, debug=False, num_devices=N_CORES)

    xp_d = nc.dram_tensor("xp", [C, L], F32, kind="ExternalInput")
    wqT_d = nc.dram_tensor("wqT", [C, C], F32R, kind="ExternalInput")
    wkT_d = nc.dram_tensor("wkT", [C, C], F32R, kind="ExternalInput")
    wvT_d = nc.dram_tensor("wvT", [C, C], F32R, kind="ExternalInput")
    owT_d = nc.dram_tensor("owT", [C, C], F32R, kind="ExternalInput")
    qb_d = nc.dram_tensor("qb", [C, 1], F32, kind="ExternalInput")
    kb_d = nc.dram_tensor("kb", [C, 1], F32, kind="ExternalInput")
    ob_d = nc.dram_tensor("ob", [C, 1], F32, kind="ExternalInput")
    gnw_d = nc.dram_tensor("gnw", [C, 1], F32, kind="ExternalInput")
    gnb_d = nc.dram_tensor("gnb", [C, 1], F32, kind="ExternalInput")
    gsel_d = nc.dram_tensor("gsel", [C, G], F32, kind="ExternalInput")
    gbro_d = nc.dram_tensor("gbro", [G, C], F32, kind="ExternalInput")
    out_d = nc.dram_tensor("out", [C, HALF], F32, kind="ExternalOutput")

    with tile.TileContext(nc) as tc:
        build_kernel(tc, xp_d, wqT_d, wkT_d, wvT_d, owT_d, qb_d, kb_d, ob_d,
                     gnw_d, gnb_d, gsel_d, gbro_d, out_d)
    return nc


def build_kernel(tc, xp_d, wqT_d, wkT_d, wvT_d, owT_d, qb_d, kb_d, ob_d,
                 gnw_d, gnb_d, gsel_d, gbro_d, out_d):
    nc = tc.nc
    fx = mybir.ActivationFunctionType
    alu = mybir.AluOpType
    ax = mybir.AxisListType

    from contextlib import ExitStack

    def T(pool, shape, tag, dtype=F32):
        return pool.tile(shape, dtype, name=tag, tag=tag)

    with ExitStack() as ctx:
        persist = ctx.enter_context(tc.tile_pool(name="persist", bufs=1))
        small = ctx.enter_context(tc.tile_pool(name="small", bufs=1))

        # ---- resident inputs -------------------------------------------------
        xp = [T(persist, [128, L], f"xp{i}") for i in range(CT)]
        for i in range(CT):
            nc.sync.dma_start(out=xp[i][:], in_=xp_d[i * 128:(i + 1) * 128, :])

        wqT = [T(small, [128, C], f"wqT{i}", F32R) for i in range(CT)]
        wkT = [T(small, [128, C], f"wkT{i}", F32R) for i in range(CT)]
        wvT = [T(small, [128, C], f"wvT{i}", F32R) for i in range(CT)]
        owT = [T(small, [128, C], f"owT{i}", F32R) for i in range(CT)]
        for i in range(CT):
            nc.sync.dma_start(out=wqT[i][:], in_=wqT_d[i * 128:(i + 1) * 128, :])
            nc.sync.dma_start(out=wkT[i][:], in_=wkT_d[i * 128:(i + 1) * 128, :])
            nc.sync.dma_start(out=wvT[i][:], in_=wvT_d[i * 128:(i + 1) * 128, :])
            nc.sync.dma_start(out=owT[i][:], in_=owT_d[i * 128:(i + 1) * 128, :])

        qb = [T(small, [128, 1], f"qb{i}") for i in range(CT)]
        kb = [T(small, [128, 1], f"kb{i}") for i in range(CT)]
        ob = [T(small, [128, 1], f"ob{i}") for i in range(CT)]
        gnw = [T(small, [128, 1], f"gnw{i}") for i in range(CT)]
        gnb = [T(small, [128, 1], f"gnb{i}") for i in range(CT)]
        for i in range(CT):
            sl = slice(i * 128, (i + 1) * 128)
            nc.sync.dma_start(out=qb[i][:], in_=qb_d[sl, :])
            nc.sync.dma_start(out=kb[i][:], in_=kb_d[sl, :])
            nc.sync.dma_start(out=ob[i][:], in_=ob_d[sl, :])
            nc.sync.dma_start(out=gnw[i][:], in_=gnw_d[sl, :])
            nc.sync.dma_start(out=gnb[i][:], in_=gnb_d[sl, :])

        gsel = [T(small, [128, G], f"gsel{i}") for i in range(CT)]
        gbro = [T(small, [G, 128], f"gbro{i}") for i in range(CT)]
        for i in range(CT):
            nc.sync.dma_start(out=gsel[i][:], in_=gsel_d[i * 128:(i + 1) * 128, :])
            nc.sync.dma_start(out=gbro[i][:], in_=gbro_d[:, i * 128:(i + 1) * 128])

        ident = T(small, [128, 128], "ident")
        make_identity(nc, ident[:])

        eps8 = T(small, [G, 1], "eps8")
        nc.vector.memset(eps8[:], EPS)

        vpad = T(small, [128, 4], "vpad")
        nc.vector.memset(vpad[:], 0.0)
        nc.vector.memset(vpad[:, 0:1], 1.0)

        # ---- phase 1: GroupNorm stats --------------------------------------
        gn = ctx.enter_context(tc.tile_pool(name="gn", bufs=2))
        gnp_cm = tc.tile_pool(name="gnp", bufs=2, space="PSUM")
        gnp = gnp_cm.__enter__()

        stats = [T(gn, [128, 2], f"stats{i}") for i in range(CT)]
        sqacc = [T(gn, [128, 8], f"sqacc{i}") for i in range(CT)]
        for i in range(CT):
            # per-channel sum over L
            nc.vector.reduce_sum(stats[i][:, 0:1], xp[i][:], axis=ax.X)
            # per-channel sum of squares, in 512-col chunks (Square's main
            # output goes to a small rotating scratch; accum_out keeps the sum)
            for j in range(8):
                scr = T(gn, [128, 512], "sq_scr")
                nc.scalar.activation(
                    scr[:], xp[i][:, j * 512:(j + 1) * 512], fx.Square,
                    accum_out=sqacc[i][:, j:j + 1],
                )
            nc.vector.reduce_sum(stats[i][:, 1:2], sqacc[i][:], axis=ax.X)

        mv8_ps = T(gnp, [G, 2], "mv8")
        for i in range(CT):
            nc.tensor.matmul(mv8_ps[:], gsel[i][:], stats[i][:],
                             start=(i == 0), stop=(i == CT - 1))
        # mean = sum/N ; E[x^2] = sumsq/N ; var = E[x^2] - mean^2
        NORM = 1.0 / ((C // G) * L)
        mv8 = T(gn, [G, 2], "mv8s")
        nc.scalar.mul(mv8[:], mv8_ps[:], NORM)
        mean8 = mv8[:, 0:1]
        ex28 = mv8[:, 1:2]
        m2 = T(gn, [G, 1], "m2")
        nc.vector.tensor_mul(m2[:], mean8, mean8)
        var8 = T(gn, [G, 1], "var8")
        nc.vector.tensor_sub(var8[:], ex28, m2[:])
        # rstd = 1/sqrt(var+eps)
        nc.scalar.activation(var8[:], var8[:], fx.Sqrt, bias=eps8[:])
        rstd8 = T(gn, [G, 1], "rstd8")
        nc.vector.reciprocal(rstd8[:], var8[:])
        grp = T(gn, [G, 2], "grp")
        nc.vector.tensor_copy(grp[:, 0:1], mean8)
        nc.vector.tensor_copy(grp[:, 1:2], rstd8[:])

        # broadcast group mean/rstd back to channels; s = gn_w*rstd,
        # t = gn_b - mean*s
        sC = [T(gn, [128, 1], f"sC{i}") for i in range(CT)]
        tC = [T(gn, [128, 1], f"tC{i}") for i in range(CT)]
        for i in range(CT):
            bc_ps = T(gnp, [128, 2], "bc")
            nc.tensor.matmul(bc_ps[:], gbro[i][:], grp[:])
            bc = T(gn, [128, 2], "bc_s")
            nc.vector.tensor_copy(bc[:], bc_ps[:])
            nc.vector.tensor_mul(sC[i][:], gnw[i][:], bc[:, 1:2])
            tmp = T(gn, [128, 1], "t_tmp")
            nc.vector.tensor_mul(tmp[:], bc[:, 0:1], sC[i][:])
            nc.vector.tensor_sub(tC[i][:], gnb[i][:], tmp[:])

        gnp_cm.__exit__(None, None, None)

        # ---- phase 2: h blocks + q/k/vT ------------------------------------
        q_sb = [T(persist, [128, HALF], f"q{i}", F32R) for i in range(CT)]
        k_sb = [T(persist, [128, L], f"k{i}", F32R) for i in range(CT)]
        vT = [T(persist, [128, C + 4], f"vT{k}", F32R) for k in range(KT)]

        with tc.tile_pool(name="hpool", bufs=3) as hpool, \
             tc.tile_pool(name="p2p", bufs=2, space="PSUM") as p2p:
            for lb in range(L // LB):
                sl = slice(lb * LB, (lb + 1) * LB)
                h = [T(hpool, [128, LB], f"h{i}", F32R) for i in range(CT)]
                for i in range(CT):
                    nc.scalar.activation(h[i][:], xp[i][:, sl], fx.Identity,
                                         bias=tC[i][:], scale=sC[i][:])
                # k block [c_out, LB]
                for co in range(CT):
                    kp = T(p2p, [128, LB], "kq_ps")
                    for ci in range(CT):
                        nc.tensor.matmul(
                            kp[:], wkT[ci][:, co * 128:(co + 1) * 128],
                            h[ci][:], start=(ci == 0), stop=(ci == CT - 1))
                    nc.scalar.activation(k_sb[co][:, sl], kp[:], fx.Identity,
                                         bias=kb[co][:])
                # q block (first half only)
                if lb < HALF // LB:
                    for co in range(CT):
                        qp = T(p2p, [128, LB], "kq_ps")
                        for ci in range(CT):
                            nc.tensor.matmul(
                                qp[:], wqT[ci][:, co * 128:(co + 1) * 128],
                                h[ci][:], start=(ci == 0), stop=(ci == CT - 1))
                        nc.scalar.activation(q_sb[co][:, sl], qp[:], fx.Identity,
                                             bias=qb[co][:])
                # vT tiles for this block: [kpos, c] = h_sliceT @ wvT
                for t in range(LB // 128):
                    kt = lb * (LB // 128) + t
                    ksl = slice(t * 128, (t + 1) * 128)
                    vp = T(p2p, [128, C], "v_ps")
                    for ci in range(CT):
                        nc.tensor.matmul(vp[:], h[ci][:, ksl], wvT[ci][:],
                                         start=(ci == 0), stop=(ci == CT - 1))
                    nc.vector.tensor_copy(vT[kt][:, 0:C], vp[:])
                    nc.scalar.copy(vT[kt][:, C:C + 4], vpad[:])

        # ---- phase 3: attention + out-proj + residual ----------------------
        with tc.tile_pool(name="expp", bufs=1) as expp, \
             tc.tile_pool(name="att", bufs=3) as att, \
             tc.tile_pool(name="scp", bufs=3, space="PSUM") as scp, \
             tc.tile_pool(name="pjp", bufs=1, space="PSUM") as pjp, \
             tc.tile_pool(name="ovp", bufs=2, space="PSUM") as ovp, \
             tc.tile_pool(name="trp", bufs=2, space="PSUM") as trp:
            for ab in range(HALF // AB):
                asl = slice(ab * AB, (ab + 1) * AB)
                expT = [T(expp, [128, AB], f"e{kt}", F32R) for kt in range(KT)]
                for kt in range(KT):
                    sc = T(scp, [128, AB], "sc")
                    for ci in range(CT):
                        nc.tensor.matmul(
                            sc[:], k_sb[ci][:, kt * 128:(kt + 1) * 128],
                            q_sb[ci][:, asl], start=(ci == 0), stop=(ci == CT - 1))
                    nc.scalar.activation(expT[kt][:], sc[:], fx.Exp,
                                         scale=float(C) ** -0.5)
                o_sb = [T(att, [128, AB], f"o{i}", F32R) for i in range(CT)]
                for lt in range(AB // 128):
                    ov = T(ovp, [128, C + 4], "ov")
                    lsl = slice(lt * 128, (lt + 1) * 128)
                    for kt in range(KT):
                        nc.tensor.matmul(ov[:], expT[kt][:, lsl], vT[kt][:],
                                         start=(kt == 0), stop=(kt == KT - 1))
                    rec = T(att, [128, 1], "rec")
                    nc.vector.reciprocal(rec[:], ov[:, C:C + 1])
                    oT = T(att, [128, C], "oT")
                    nc.vector.tensor_scalar_mul(oT[:], ov[:, 0:C], rec[:])
                    for i in range(CT):
                        tr = T(trp, [128, 128], "tr")
                        nc.tensor.transpose(tr[:], oT[:, i * 128:(i + 1) * 128],
                                            ident[:])
                        nc.scalar.copy(o_sb[i][:, lsl], tr[:])
                # out proj + residual
                for co in range(CT):
                    pj = T(pjp, [128, AB], "pj")
                    for ci in range(CT):
                        nc.tensor.matmul(
                            pj[:], owT[ci][:, co * 128:(co + 1) * 128],
                            o_sb[ci][:], start=(ci == 0), stop=(ci == CT - 1))
                    y = T(att, [128, AB], f"y{co}")
                    nc.vector.tensor_add(y[:], pj[:], xp[co][:, asl])
                    nc.vector.tensor_scalar_add(y[:], y[:], ob[co][:])
                    nc.sync.dma_start(out=out_d[co * 128:(co + 1) * 128, asl],
                                      in_=y[:])


_NC_CACHE = {}


def _get_nc():
    if "nc" not in _NC_CACHE:
        _patch_tile_drain()
        _NC_CACHE["nc"] = build_nc()
    return _NC_CACHE["nc"]


def make_in_maps(x, gn_w, gn_b, qkv_w, qkv_b, out_w, out_b):
    """Build the 8 per-core input maps from the full problem inputs."""
    x = np.asarray(x, np.float32)
    gn_w = np.asarray(gn_w, np.float32)
    gn_b = np.asarray(gn_b, np.float32)
    qkv_w = np.asarray(qkv_w, np.float32)
    qkv_b = np.asarray(qkv_b, np.float32)
    out_w = np.asarray(out_w, np.float32)
    out_b = np.asarray(out_b, np.float32)

    wqT = np.ascontiguousarray(qkv_w[0:C].T)
    wkT = np.ascontiguousarray(qkv_w[C:2 * C].T)
    wvT = np.ascontiguousarray(qkv_w[2 * C:3 * C].T)
    owT = np.ascontiguousarray(out_w.T)
    qb = qkv_w[0:C, 0:1] * 0 + qkv_b[0:C, None]
    kb = qkv_b[C:2 * C, None]
    # v-bias folds through attention (rows of attn sum to 1) into the
    # output projection: ob_eff = out_w @ v_b + out_b
    ob = (out_w @ qkv_b[2 * C:3 * C] + out_b)[:, None]
    gnw = gn_w[:, None]
    gnb = gn_b[:, None]

    ch = np.arange(C)
    gsel = np.zeros((C, G), np.float32)
    gsel[ch, ch // (C // G)] = 1.0
    gbro = np.ascontiguousarray(gsel.T)

    common = dict(wqT=wqT, wkT=wkT, wvT=wvT, owT=owT,
                  qb=np.ascontiguousarray(qb, np.float32),
                  kb=np.ascontiguousarray(kb, np.float32),
                  ob=np.ascontiguousarray(ob, np.float32),
                  gnw=np.ascontiguousarray(gnw), gnb=np.ascontiguousarray(gnb),
                  gsel=gsel, gbro=gbro)

    in_maps = []
    for core in range(N_CORES):
        b, half = divmod(core, 2)
        if half == 0:
            xp = x[b]
        else:
            xp = np.concatenate([x[b][:, HALF:], x[b][:, :HALF]], axis=1)
        in_maps.append({"xp": np.ascontiguousarray(xp), **common})
    return in_maps


def assemble_output(results, x_dtype):
    y = np.empty((B, C, L), np.float32)
    for core in range(N_CORES):
        b, half = divmod(core, 2)
        y[b][:, half * HALF:(half + 1) * HALF] = results[core]["out"]
    return y.astype(x_dtype, copy=False)


def kernel(x, gn_w, gn_b, qkv_w, qkv_b, out_w, out_b):
    from concourse.bass_utils import run_bass_kernel_spmd

    nc = _get_nc()
    in_maps = make_in_maps(x, gn_w, gn_b, qkv_w, qkv_b, out_w, out_b)
    res = run_bass_kernel_spmd(nc, in_maps, core_ids=list(range(N_CORES)))
    return assemble_output(res.results, np.asarray(x).dtype)
